# revision 15
# baseline (speedup 1.0000x reference)
"""DIN attention kernel for Trainium2 (8 NeuronCores, data-parallel over batch).

Math (per batch row b, position s):
  din  = [t, seq, t-seq, t*seq]  -> relu MLP 256->80->40->1 -> masked softmax over s.

Key structure:
- Weight fold: din @ W1 = seq @ Ws' + (t*seq) @ Wm + U with
  U = t @ Wt' + b1 per row. The per-row U term is folded into the
  shipped data on the host: solve Wsm^T z_row = U_row (128 unknowns,
  80 equations, least-norm via (Wsm^T Wsm)^-1 computed against the
  bf16-rounded device weights) and add z_row to every gathered column
  before the bf16 cast, so mm1 produces ps1 + U directly and the device
  does no U work at all.
- The elementwise product t*seq is precomputed on the host during the
  gather, so din = [seq+z1; t*seq+z2] arrives via two plain 2D DMAs.
- Mask sparsity: max unmasked count per row < 128, so each batch row is
  gathered on host to exactly 128 feature-major columns (bf16). Padding
  columns get an additive -1e9 mask so they softmax to exactly 0; the
  host scatters probabilities back to the full [B, 200] grid.
- On-device per 16-row chunk: mm1 (K=128, M=80) into two [80,1024] PSUM
  tiles (2 banks each) so h1 relu is 2 scalar ACTs, mm2 2-way
  column-tiled into [128,512] pair tiles, h2 bias+relu on the vector
  engine, and M=32 shifted-column w3 matmuls (8-way tiled) accumulating
  scores into [128,128] PSUM tiles per 128-row superblock where the
  masked softmax runs in natural layout.
- Batch rows are processed in a permuted order (dev row 4r+q <-> score
  row 32q+r); the host permutation arrays and the final scatter undo it.
"""

import sys

sys.path.insert(0, "/opt/trn_rl_repo")

import numpy as np
import ml_dtypes

B, S, D = 4096, 200, 64
H1, H2 = 80, 40
NCORES = 8
BPC = B // NCORES          # 512 batch rows per core
CAP = 128                  # gathered positions per batch row
CHUNK_B = 16               # batch rows per chunk (4 quads)
NCHUNK = BPC // CHUNK_B    # 32
NSUPER = BPC // 128        # 4 superblocks (128 rows each)

_cache = {}


def _build_nc():
    import concourse.bass as bass
    import concourse.mybir as mybir
    import concourse.tile as tile
    from concourse import bacc

    f32 = mybir.dt.float32
    bf16 = mybir.dt.bfloat16
    AF = mybir.ActivationFunctionType
    ALU = mybir.AluOpType

    nc = bacc.Bacc(None, target_bir_lowering=False)

    seq_d = nc.declare_dram_parameter("seqg", [NCHUNK, D, CHUNK_B * CAP], bf16, isOutput=False)
    prod_d = nc.declare_dram_parameter("prodg", [NCHUNK, D, CHUNK_B * CAP], bf16, isOutput=False)
    wsm_d = nc.declare_dram_parameter("wsm", [2 * D, H1], bf16, isOutput=False)
    w2_d = nc.declare_dram_parameter("w2", [H1, 64], bf16, isOutput=False)
    w3s_d = nc.declare_dram_parameter("w3s2", [128, 64], bf16, isOutput=False)
    b2_d = nc.declare_dram_parameter("b2e", [128, 1], f32, isOutput=False)
    am_d = nc.declare_dram_parameter("amaskf", [BPC, CAP], f32, isOutput=False)
    out_d = nc.declare_dram_parameter("out", [BPC, CAP], f32, isOutput=True)

    with tile.TileContext(nc) as tc:
        with (
            tc.tile_pool(name="singles", bufs=1) as singles,
            tc.tile_pool(name="dinpool", bufs=3) as dinpool,
            tc.tile_pool(name="h1pool", bufs=4) as h1pool,
            tc.tile_pool(name="h2pool", bufs=4) as h2pool,
            tc.tile_pool(name="smpool", bufs=2) as smpool,
            tc.tile_pool(name="ps1pool", bufs=1, space="PSUM") as ps1pool,
            tc.tile_pool(name="ps2pool", bufs=2, space="PSUM") as ps2pool,
            tc.tile_pool(name="scpool", bufs=1, space="PSUM") as scpool,
            tc.tile_pool(name="scpoolb", bufs=1, space="PSUM") as scpoolb,
        ):
            # prefetch chunk 0 before the (larger) weight singles
            din0 = dinpool.tile([128, CHUNK_B, CAP], bf16, tag="din")
            nc.sync.dma_start(
                out=din0[0:D, :, :],
                in_=seq_d[0, :, :].rearrange("d (k t) -> d k t", k=CHUNK_B))
            nc.gpsimd.dma_start(
                out=din0[D:128, :, :],
                in_=prod_d[0, :, :].rearrange("d (k t) -> d k t", k=CHUNK_B))
            wsm = singles.tile([2 * D, H1], bf16)
            nc.sync.dma_start(out=wsm, in_=wsm_d[:])
            w2 = singles.tile([H1, 64], bf16)
            nc.sync.dma_start(out=w2, in_=w2_d[:])
            w3s2 = singles.tile([128, 64], bf16)
            nc.sync.dma_start(out=w3s2, in_=w3s_d[:])
            b2e = singles.tile([128, 1], f32)
            nc.sync.dma_start(out=b2e, in_=b2_d[:])

            # HAM warm-up: the PE clock-gate only opens (1.2 -> 2.4 GHz) after
            # ~3.4us of sustained matmul activity, and the first real matmul
            # waits ~6us on the input DMA pipe anyway. Fill that window with
            # dummy matmuls into the sb0 score tiles (whose first real writes
            # use start=True, so the garbage is never observed).
            zt = singles.tile([128, 512], bf16)
            nc.vector.memset(zt, 0.0)
            sc_tiles = {}
            sc_tiles[0] = (scpool.tile([128, CAP], f32, name="scA"),
                           scpoolb.tile([128, CAP], f32, name="scB"))
            for w in range(40):
                nc.tensor.matmul(
                    sc_tiles[0][w % 2], lhsT=zt[:, 0:128], rhs=zt[:, 0:CAP],
                    start=True, stop=(w >= 38), skip_group_check=True)

            # software-pipelined: per chunk c emit mm1/relu/mm2/h2 for c and
            # the score matmuls for c-1, so the in-order tensor queue is
            # [mm1 x4, mm3 x16 (prev), mm2 x4] and never stalls on the relu.
            h2q = {}

            def emit_mm3(c):
                sb, ch = c // 8, c % 8
                scA, scB = sc_tiles[sb]
                for pair in range(2):
                    h2t = h2q.pop((c, pair))
                    ra = ch * 4 + 2 * pair
                    first = (ch == 0 and pair == 0)
                    last = (ch == 7 and pair == 1)
                    for q in range(4):
                        nc.tensor.matmul(
                            scA[32 * q:32 * q + 32, :],
                            lhsT=w3s2[0:H2, 32 - ra:64 - ra],
                            rhs=h2t[0:H2, q * CAP:(q + 1) * CAP],
                            start=first, stop=last,
                            tile_position=(0, 32 * q),
                            skip_group_check=True)
                        nc.tensor.matmul(
                            scB[32 * q:32 * q + 32, :],
                            lhsT=w3s2[64:64 + H2, 31 - ra:63 - ra],
                            rhs=h2t[64:64 + H2, q * CAP:(q + 1) * CAP],
                            start=first, stop=last,
                            tile_position=(64, 32 * q),
                            skip_group_check=True)

            def emit_softmax(sb):
                scA, scB = sc_tiles.pop(sb)
                amt = amts.pop(sb)
                scm = smpool.tile([128, CAP], f32)
                nc.vector.tensor_add(scm, scA, amt)
                scm2 = smpool.tile([128, CAP], f32)
                nc.vector.tensor_add(scm2, scm, scB)
                negmax = smpool.tile([128, 1], f32)
                nc.vector.tensor_reduce(
                    negmax, scm2, axis=mybir.AxisListType.X,
                    op=ALU.max, negate=True)
                expm = smpool.tile([128, CAP], f32)
                sume = smpool.tile([128, 1], f32)
                nc.scalar.activation(expm, scm2, AF.Exp,
                                     bias=negmax[:, 0:1], accum_out=sume)
                rec = smpool.tile([128, 1], f32)
                nc.vector.reciprocal(rec, sume)
                outt = smpool.tile([128, CAP], f32)
                nc.vector.tensor_scalar_mul(outt, expm, rec[:, 0:1])
                nc.sync.dma_start(out=out_d[sb * 128:(sb + 1) * 128, :],
                                  in_=outt)

            amts = {}
            for c in range(NCHUNK):
                sb, ch = c // 8, c % 8
                if ch == 0:
                    if sb not in sc_tiles:
                        sc_tiles[sb] = (scpool.tile([128, CAP], f32, name="scA"),
                                        scpoolb.tile([128, CAP], f32, name="scB"))
                    amt = smpool.tile([128, CAP], f32)
                    nc.sync.dma_start(
                        out=amt, in_=am_d[sb * 128:(sb + 1) * 128, :])
                    amts[sb] = amt
                if c == 0:
                    din = din0
                else:
                    din = dinpool.tile([128, CHUNK_B, CAP], bf16, tag="din")
                    nc.sync.dma_start(
                        out=din[0:D, :, :],
                        in_=seq_d[c, :, :].rearrange("d (k t) -> d k t", k=CHUNK_B))
                    nc.gpsimd.dma_start(
                        out=din[D:128, :, :],
                        in_=prod_d[c, :, :].rearrange("d (k t) -> d k t", k=CHUNK_B))

                # mm1 into two [80, 2x512] PSUM tiles (2 banks each)
                ps1a = ps1pool.tile([H1, 2, 512], f32)
                ps1b = ps1pool.tile([H1, 2, 512], f32)
                ps1s = [ps1a, ps1a, ps1b, ps1b]
                for qq in range(4):
                    nc.tensor.matmul(
                        ps1s[qq][:, qq % 2, :],
                        lhsT=wsm, rhs=din[:, 4 * qq:4 * qq + 4, :],
                        start=True, stop=True, skip_group_check=True)
                h1s = []
                for half, ps1 in enumerate((ps1a, ps1b)):
                    h1t = h1pool.tile([H1, 2, 512], bf16, tag=f"h1{half}")
                    nc.scalar.activation(h1t, ps1, AF.Relu)
                    h1s += [h1t, h1t]

                if c > 0:
                    emit_mm3(c - 1)
                    if ch == 0:
                        emit_softmax(sb - 1)

                # mm2: pairs (q0,q1)->ps2a rows {0-39,64-103}, (q2,q3)->ps2b
                ps2s = []
                for qq in range(4):
                    rb = 0 if qq % 2 == 0 else 64
                    if qq % 2 == 0:
                        ps2 = ps2pool.tile([128, 512], f32)
                        ps2s.append(ps2)
                    nc.tensor.matmul(ps2s[-1][rb:rb + H2, :],
                                     lhsT=w2[:, 0:H2],
                                     rhs=h1s[qq][:, qq % 2, :],
                                     start=True, stop=True,
                                     tile_position=(0, rb))
                for pair in range(2):
                    h2t = h2pool.tile([128, 512], bf16, tag="h2")
                    nc.vector.tensor_scalar(
                        h2t[0:104, :], ps2s[pair][0:104, :],
                        b2e[0:104, 0:1], 0.0,
                        op0=ALU.add, op1=ALU.max)
                    h2q[(c, pair)] = h2t

            emit_mm3(NCHUNK - 1)
            emit_softmax(NSUPER - 1)

    nc.finalize()
    return nc


def _host_prep(inputs):
    bf16 = ml_dtypes.bfloat16
    seq = np.asarray(inputs["sequence_emb"], dtype=np.float32)
    tgt = np.asarray(inputs["target_emb"], dtype=np.float32)
    mask = np.asarray(inputs["mask"])
    W1 = np.asarray(inputs["W1"], dtype=np.float32)
    b1 = np.asarray(inputs["b1"], dtype=np.float32)
    W2 = np.asarray(inputs["W2"], dtype=np.float32)
    b2 = np.asarray(inputs["b2"], dtype=np.float32)
    W3 = np.asarray(inputs["W3"], dtype=np.float32)

    Wt = W1[0:64] + W1[128:192]
    Ws = W1[64:128] - W1[128:192]
    Wm = W1[192:256]
    wsm = np.concatenate([Ws, Wm], axis=0).astype(bf16)
    # Fold U = t @ Wt' + b1 into the shipped data: solve wsm^T z = U
    # (least-norm, against the bf16-rounded device weights).
    wsm_f = wsm.astype(np.float32)
    Zmap = (wsm_f @ np.linalg.inv(wsm_f.T @ wsm_f)).astype(np.float32)  # [128, H1]
    U = tgt @ Wt + b1                                   # [B, H1] f32
    Z = U @ Zmap.T                                      # [B, 128]
    w2 = np.zeros((H1, 64), dtype=bf16)
    w2[:, 0:H2] = W2.astype(bf16)
    w3s2 = np.zeros((128, 64), dtype=bf16)
    w3s2[0:H2, 32] = W3[:, 0].astype(bf16)
    w3s2[64:64 + H2, 32] = W3[:, 0].astype(bf16)
    b2e = np.zeros((128, 1), dtype=np.float32)
    b2e[0:H2, 0] = b2
    b2e[64:64 + H2, 0] = b2

    maskb = mask.astype(bool)
    cnt = maskb.sum(1).astype(np.int64)
    assert cnt.max() <= CAP, f"unmasked count {cnt.max()} exceeds CAP={CAP}"
    # gather indices: unmasked positions first, padded with a masked slot
    order = np.argsort(~maskb, axis=1, kind="stable")   # unmasked first
    idx = order[:, :CAP]
    pad_slot = order[:, -1]                             # guaranteed masked
    colpos = np.arange(CAP)[None, :]
    idx = np.where(colpos < cnt[:, None], idx, pad_slot[:, None])

    # device row permutation within each 128-superblock: dev 4r+q <-> 32q+r
    r_ = np.arange(128) // 4
    q_ = np.arange(128) % 4
    perm128 = 32 * q_ + r_
    perm = np.concatenate(
        [sb * 128 + perm128 for sb in range(B // 128)])      # [B] dev->orig

    # gathered feature-major seq+z1 and t*seq+z2, device row order,
    # chunk-major bf16: [B/16, 64, 16*128], one plain 2D DMA per chunk
    def chunk_major(x):                                  # [B, CAP, D] f32
        return np.ascontiguousarray(
            x.transpose(0, 2, 1)[perm.reshape(B)]        # [B, D, CAP]
            .reshape(B // CHUNK_B, CHUNK_B, D, CAP)
            .transpose(0, 2, 1, 3)
            .reshape(B // CHUNK_B, D, CHUNK_B * CAP)).astype(bf16)

    g = np.take_along_axis(seq, idx[:, :, None], axis=1)     # [B, CAP, D]
    seq_g = chunk_major(g + Z[:, None, 0:64])
    prod_g = chunk_major(g * tgt[:, None, :] + Z[:, None, 64:128])

    amask = np.where(colpos < cnt[:, None], 0.0, -1e9).astype(np.float32)  # original row order

    in_maps = []
    for core in range(NCORES):
        b0 = core * BPC
        in_maps.append({
            "seqg": seq_g[b0 // CHUNK_B:(b0 + BPC) // CHUNK_B],
            "prodg": prod_g[b0 // CHUNK_B:(b0 + BPC) // CHUNK_B],
            "wsm": wsm,
            "w2": w2,
            "w3s2": w3s2,
            "b2e": b2e,
            "amaskf": amask[b0:b0 + BPC],
        })
    return in_maps, idx, perm


def kernel(**inputs) -> np.ndarray:
    from concourse.bass_utils import run_bass_kernel_spmd

    if "nc" not in _cache:
        _cache["nc"] = _build_nc()
    nc = _cache["nc"]
    in_maps, idx, perm = _host_prep(inputs)
    res = run_bass_kernel_spmd(nc, in_maps, list(range(NCORES)))
    probs = np.concatenate(
        [res.results[i]["out"] for i in range(NCORES)], axis=0)  # [B, CAP], original row order
    out = np.zeros((B, S), dtype=np.float32)
    rows = np.arange(B)
    out[rows[:, None], idx] = probs
    return out


if __name__ == "__main__":
    rng = np.random.default_rng(0)
    fake = {
        "sequence_emb": rng.standard_normal((B, S, D), dtype=np.float32),
        "target_emb": rng.standard_normal((B, D), dtype=np.float32),
        "mask": rng.integers(0, 2, (B, S)).astype(np.int32),
        "W1": rng.standard_normal((4 * D, H1), dtype=np.float32) * 0.08,
        "b1": np.zeros(H1, np.float32),
        "W2": rng.standard_normal((H1, H2), dtype=np.float32) * 0.13,
        "b2": np.zeros(H2, np.float32),
        "W3": rng.standard_normal((H2, 1), dtype=np.float32) * 0.22,
        "b3": np.zeros(1, np.float32),
    }
    print(kernel(**fake).shape)
